# revision 1
# baseline (speedup 1.0000x reference)
"""Multi-head attention (B=1, S=2048, D=2048, H=16, d_k=128) on 8 Trainium2
NeuronCores via Bass/Tile.

Sharding: tensor-parallel over heads. Each core owns 2 heads: it gets the
column shards of Wq/Wk/Wv and the row shard of Wo for those heads, computes
its partial output projection, and the host sums the 8 partials (the
all-reduce equivalent) and adds biases.

All matmuls run in fp16 (single-pass, full PE rate; fp32 PSUM
accumulation). Measured end-to-end relative error ~1e-3 against the fp32
reference, dominated by fp16 rounding of x/W/Q/K.

Per-core dataflow (everything derived from x^T so contractions sit on the
partition axis):
  phase 1: Q^T = Wq_s.T @ x^T-chunks, K^T likewise, V = x @ Wv_s (natural)
  phase 2: per (head, q-chunk): S^T[k,q] = K^T.T @ Q^T (one 128-contraction
           matmul per k-tile), ACT exp -> P^T (f32r), then two accumulating
           matmuls over the 16 k-tiles: ones-lhsT -> softmax denominator
           (broadcast across partitions) and V-lhsT -> unnormalized O^T.
           DVE reciprocal+multiply normalizes.
  phase 3: out_partial[q,e] = O^T.T @ Wo_s, accumulated over the 2 d-tiles.
"""

import sys

sys.path.insert(0, "/opt/trn_rl_repo")

import numpy as np

S = 2048
D = 2048
H = 16
DK = 128
N_CORES = 8
HEADS_PER_CORE = H // N_CORES  # 2
DPC = HEADS_PER_CORE * DK  # 256, per-core projection width
SCALE = 1.0 / np.sqrt(DK)

TRACE = False  # test.py flips this to get an NTFF profile + exec time
_LAST_EXEC_NS = [None]
_LAST_RESULTS = [None]

_PROGRAM = [None]


def _build_program():
    from concourse import bacc, bass_isa, mybir
    from concourse.tile import TileContext

    f32 = mybir.dt.float32
    f32r = mybir.dt.float32r
    f16 = mybir.dt.float16

    nc = bacc.Bacc()
    xT = nc.declare_dram_parameter("xT", [D, S], f16, isOutput=False)
    wq = nc.declare_dram_parameter("wq", [D, DPC], f16, isOutput=False)
    wk = nc.declare_dram_parameter("wk", [D, DPC], f16, isOutput=False)
    wv = nc.declare_dram_parameter("wv", [D, DPC], f16, isOutput=False)
    wo = nc.declare_dram_parameter("wo", [DPC, S], f16, isOutput=False)
    out = nc.declare_dram_parameter("out", [S, D], f32, isOutput=True)

    ND = D // 128  # 16 d-tiles of the model dim
    NS = S // 128  # 16 s-tiles
    NQ = S // 512  # 4 q/s chunks
    EXP = mybir.ActivationFunctionType.Exp
    CPY = mybir.ActivationFunctionType.Copy

    xT_r = xT[:].rearrange("(t p) s -> p t s", p=128)  # [128, 16, 2048]
    wq_r = wq[:].rearrange("(t p) n -> p t n", p=128)  # [128, 16, 256]
    wk_r = wk[:].rearrange("(t p) n -> p t n", p=128)
    wv_r = wv[:].rearrange("(t p) n -> p t n", p=128)
    wo_r = wo[:].rearrange("(t p) e -> p t e", p=128)  # [128, 2, 2048]

    with TileContext(nc) as tc:
        with (
            tc.tile_pool(name="wpool", bufs=1) as wpool,
            tc.tile_pool(name="xpool", bufs=3) as xpool,
            tc.tile_pool(name="qkv", bufs=1) as qkv,
            tc.tile_pool(name="ppool", bufs=3) as ppool,
            tc.tile_pool(name="rpool", bufs=1) as rpool,
            tc.tile_pool(name="obpool", bufs=3) as obpool,
            tc.tile_pool(name="psA", bufs=2, space="PSUM") as psA,
            tc.tile_pool(name="psB", bufs=2, space="PSUM") as psB,
            tc.tile_pool(name="psC", bufs=2, space="PSUM") as psC,
        ):
            # resident weights (qkv projections); wo rides in an xpool slot
            # after the x^T stream is done with it.
            wq_sb = wpool.tile([128, ND * DPC], f16, tag="wq")
            wk_sb = wpool.tile([128, ND * DPC], f16, tag="wk")
            wv_sb = wpool.tile([128, ND * DPC], f16, tag="wv")

            # per-head Q^T/K^T [128, S] and V in natural layout [128, NS*DPC]
            qt_sb = [qkv.tile([128, S], f16, tag=f"qt{h}", name=f"qt{h}") for h in range(2)]
            kt_sb = [qkv.tile([128, S], f16, tag=f"kt{h}", name=f"kt{h}") for h in range(2)]
            v_sb = qkv.tile([128, NS * DPC], f16, tag="v")

            # ---------------- phase 1: projections ----------------
            for sc in range(NQ):  # 4 chunks of 512 seq positions
                xt = xpool.tile([128, ND * 512], f16, tag="xt")
                if sc == 0:
                    # interleaved per-d-tile DMAs: the d-tile-i matmul of the
                    # first Q^T/K^T accumulation only waits for slice i, so PE
                    # starts ~2us in instead of after the full 8MB prefetch
                    for dt_ in range(ND):
                        nc.sync.dma_start(
                            out=wq_sb[:, dt_ * DPC:(dt_ + 1) * DPC],
                            in_=wq_r[:, dt_, :],
                        )
                        nc.sync.dma_start(
                            out=xt[:, dt_ * 512:(dt_ + 1) * 512],
                            in_=xT_r[:, dt_, 0:512],
                        )
                    for dt_ in range(ND):
                        nc.sync.dma_start(
                            out=wk_sb[:, dt_ * DPC:(dt_ + 1) * DPC],
                            in_=wk_r[:, dt_, :],
                        )
                else:
                    for dt_ in range(ND):
                        nc.sync.dma_start(
                            out=xt[:, dt_ * 512:(dt_ + 1) * 512],
                            in_=xT_r[:, dt_, sc * 512:(sc + 1) * 512],
                        )
                # Q^T and K^T: [n_tile 128, s 512] = sum_d W[d, n].T @ xT[d, s]
                for w_sb, dst in ((wq_sb, qt_sb), (wk_sb, kt_sb)):
                    for h in range(2):
                        ps = psA.tile([128, 512], f32, tag="proj")
                        for dt_ in range(ND):
                            nc.tensor.matmul(
                                ps[:],
                                w_sb[:, dt_ * DPC + h * 128: dt_ * DPC + h * 128 + 128],
                                xt[:, dt_ * 512:(dt_ + 1) * 512],
                                start=(dt_ == 0),
                                stop=(dt_ == ND - 1),
                            )
                        nc.vector.tensor_copy(
                            dst[h][:, sc * 512:(sc + 1) * 512], ps[:]
                        )
                    if sc == 0 and w_sb is wq_sb:
                        # wv arrives while the first K^T chunk computes
                        nc.sync.dma_start(
                            out=wv_sb[:].rearrange("p (t n) -> p t n", n=DPC),
                            in_=wv_r,
                        )
                # V natural: [s_tile 128, 256] = sum_d xT[d, s_tile].T @ Wv[d, :]
                for st in range(4):
                    s_tile = sc * 4 + st
                    ps = psA.tile([128, 512], f32, tag="proj")
                    for dt_ in range(ND):
                        nc.tensor.matmul(
                            ps[:, 0:DPC],
                            xt[:, dt_ * 512 + st * 128: dt_ * 512 + st * 128 + 128],
                            wv_sb[:, dt_ * DPC:(dt_ + 1) * DPC],
                            start=(dt_ == 0),
                            stop=(dt_ == ND - 1),
                        )
                    nc.vector.tensor_copy(
                        v_sb[:, s_tile * DPC:(s_tile + 1) * DPC], ps[:, 0:DPC]
                    )

            # wo reuses a retired x^T-stream slot (same tag/shape)
            wo_ot = xpool.tile([128, ND * 512], f16, tag="xt")
            wo_sb = wo_ot[:, 0:2 * S]
            nc.sync.dma_start(
                out=wo_sb.rearrange("p (t e) -> p t e", e=S), in_=wo_r
            )

            # ------- phases 2+3 interleaved per q-chunk -------
            # attention for both heads of a q-chunk; the chunk's
            # output-projection rows are emitted one chunk later so they
            # overlap the next chunk's attention instead of stalling on the
            # O^T normalize at the chunk boundary
            # O^T normalized output, one [128, 512] tile per (head, q-chunk);
            # separate tiles (not slices of one tensor) so the delayed
            # projection reads don't pick up false deps on later chunks'
            # writes. 6 slots: 2 chunks * 2 heads live + write-ahead.
            ot_tiles = {}

            def emit_proj(qc_done):
                for qt_ in range(qc_done * 4, qc_done * 4 + 4):
                    for ec in range(NQ):
                        ps = psA.tile([128, 512], f32, tag="proj", name="proj_ps")
                        for dt_ in range(2):
                            nc.tensor.matmul(
                                ps[:],
                                ot_tiles[(dt_, qc_done)][:, (qt_ - qc_done * 4) * 128:(qt_ - qc_done * 4 + 1) * 128],
                                wo_sb[:, dt_ * S + ec * 512: dt_ * S + ec * 512 + 512],
                                start=(dt_ == 0),
                                stop=(dt_ == 1),
                            )
                        ob = obpool.tile([128, 512], f32, tag="ob", name="ob")
                        if ec % 2 == 0:
                            nc.scalar.activation(ob[:], ps[:], CPY)
                        else:
                            nc.vector.tensor_copy(ob[:], ps[:])
                        nc.sync.dma_start(
                            out=out[qt_ * 128:(qt_ + 1) * 128, ec * 512:(ec + 1) * 512],
                            in_=ob[:],
                        )

            for qc in range(NQ):
                for h in range(2):
                    if h == 1 and qc > 1:
                        emit_proj(qc - 2)
                    oT = psB.tile([128, 512], f32, tag="oT", bufs=3, name="oT")
                    qt_slice = qt_sb[h][:, qc * 512:(qc + 1) * 512]
                    st_tiles = {}
                    st_tiles[0] = psC.tile([128, 512], f32, tag="st", name="st0", bufs=3)
                    nc.tensor.matmul(
                        st_tiles[0][:], kt_sb[h][:, 0:128], qt_slice,
                        start=True, stop=True,
                    )
                    # softmax denominator: DVE-accumulate the exp tiles, then
                    # one GPSIMD cross-partition all-reduce (broadcast result)
                    acc = rpool.tile([128, 512], f32, tag="acc", bufs=2, name="acc")
                    for kt_ in range(NS):
                        pt = ppool.tile([128, 512], f16, tag="pt")
                        nc.scalar.activation(
                            pt[:], st_tiles.pop(kt_)[:], EXP, scale=float(SCALE)
                        )
                        if kt_ + 1 < NS:
                            st_tiles[kt_ + 1] = psC.tile([128, 512], f32, tag="st", name="stn", bufs=3)
                            nc.tensor.matmul(
                                st_tiles[kt_ + 1][:],
                                kt_sb[h][:, (kt_ + 1) * 128:(kt_ + 2) * 128],
                                qt_slice,
                                start=True, stop=True,
                            )
                        if kt_ == 0:
                            nc.vector.tensor_copy(acc[:], pt[:])
                        else:
                            nc.vector.tensor_add(acc[:], acc[:], pt[:])
                        nc.tensor.matmul(
                            oT[:],
                            v_sb[:, kt_ * DPC + h * 128: kt_ * DPC + h * 128 + 128],
                            pt[:],
                            start=(kt_ == 0), stop=(kt_ == NS - 1),
                        )
                    bc = rpool.tile([128, 512], f32, tag="bc", bufs=2, name="bc")
                    nc.gpsimd.partition_all_reduce(
                        bc[:], acc[:], 128, bass_isa.ReduceOp.add
                    )
                    rc = rpool.tile([128, 512], f32, tag="recip")
                    # ~18-bit 1/x, ~5x faster than exact reciprocal; softmax
                    # denominators are well-conditioned positives (~1e2..4e3)
                    nc.vector.reciprocal_approx_fast(rc[:], bc[:])
                    ot_tiles[(h, qc)] = ppool.tile(
                        [128, 512], f16, tag="ot", bufs=8, name="ot_t"
                    )
                    nc.vector.tensor_mul(ot_tiles[(h, qc)][:], oT[:], rc[:])
            emit_proj(NQ - 2)
            emit_proj(NQ - 1)

    nc.compile()
    return nc


def _numpy_fallback(x, mask, Wq, bq, Wk, bk, Wv, bv, Wo, bo):
    B, S_, D_ = x.shape
    xf = x.reshape(S_, D_).astype(np.float64)

    def proj(W, b):
        y = xf @ W.astype(np.float64) + b.astype(np.float64)
        return y.reshape(S_, H, DK).transpose(1, 0, 2)

    Q = proj(Wq, bq)
    K = proj(Wk, bk)
    V = proj(Wv, bv)
    m = np.broadcast_to(mask, (B, H, S_, S_))
    out = np.empty((H, S_, DK))
    for h in range(H):
        sc = Q[h] @ K[h].T / np.sqrt(DK)
        sc = np.where(m[0, h], sc, -np.inf)
        sc -= sc.max(axis=-1, keepdims=True)
        e = np.exp(sc)
        p = e / e.sum(axis=-1, keepdims=True)
        out[h] = p @ V[h]
    o = out.transpose(1, 0, 2).reshape(S_, D_)
    res = o @ Wo.astype(np.float64) + bo.astype(np.float64)
    return res.reshape(B, S_, D_).astype(np.float32)


def kernel(x, mask, Wq, bq, Wk, bk, Wv, bv, Wo, bo):
    x = np.asarray(x, dtype=np.float32)
    mask = np.asarray(mask)
    Wq = np.asarray(Wq, dtype=np.float32)
    Wk = np.asarray(Wk, dtype=np.float32)
    Wv = np.asarray(Wv, dtype=np.float32)
    Wo = np.asarray(Wo, dtype=np.float32)
    bq = np.asarray(bq, dtype=np.float32)
    bk = np.asarray(bk, dtype=np.float32)
    bv = np.asarray(bv, dtype=np.float32)
    bo = np.asarray(bo, dtype=np.float32)

    # Off-benchmark shapes/masks/biases: exact numpy fallback.
    # (bk shifts every score row by a constant -> softmax-invariant; bv and bo
    # are affine in the output and folded in on the host; only bq actually
    # changes the attention pattern in a way the device kernel doesn't model.)
    if x.shape != (1, S, D) or not bool(mask.all()) or np.any(bq):
        return _numpy_fallback(x, mask, Wq, bq, Wk, bk, Wv, bv, Wo, bo)

    from concourse.bass_utils import run_bass_kernel_spmd

    if _PROGRAM[0] is None:
        _PROGRAM[0] = _build_program()
    nc = _PROGRAM[0]

    xT = np.ascontiguousarray(x.reshape(S, D).T.astype(np.float16))
    in_maps = []
    for c in range(N_CORES):
        lo, hi = c * DPC, (c + 1) * DPC
        in_maps.append(
            {
                "xT": xT,
                "wq": np.ascontiguousarray(Wq[:, lo:hi].astype(np.float16)),
                "wk": np.ascontiguousarray(Wk[:, lo:hi].astype(np.float16)),
                "wv": np.ascontiguousarray(Wv[:, lo:hi].astype(np.float16)),
                "wo": np.ascontiguousarray(Wo[lo:hi, :].astype(np.float16)),
            }
        )

    res = run_bass_kernel_spmd(nc, in_maps, list(range(N_CORES)), trace=TRACE)
    _LAST_EXEC_NS[0] = res.exec_time_ns
    _LAST_RESULTS[0] = res

    acc = res.results[0]["out"].astype(np.float64)
    for c in range(1, N_CORES):
        acc += res.results[c]["out"]
    # bv contributes (attn rows sum to 1) a constant bv @ Wo; bo is additive.
    acc += (bv.astype(np.float64) @ Wo) + bo
    return acc.astype(np.float32).reshape(1, S, D)



# revision 2
# speedup vs baseline: 1.1201x; 1.1201x over previous
"""Multi-head attention (B=1, S=2048, D=2048, H=16, d_k=128) on 8 Trainium2
NeuronCores via Bass/Tile.

Sharding: tensor-parallel over heads. Each core owns 2 heads: it gets the
column shards of Wq/Wk/Wv and the row shard of Wo for those heads, computes
its partial output projection, and the host sums the 8 partials (the
all-reduce equivalent) and adds biases.

All matmuls run in fp16 (single-pass, full PE rate; fp32 PSUM
accumulation). Measured end-to-end relative error ~1e-3 against the fp32
reference, dominated by fp16 rounding of x/W/Q/K.

Per-core dataflow (everything derived from x^T so contractions sit on the
partition axis):
  phase 1: Q^T = Wq_s.T @ x^T-chunks, K^T likewise, V = x @ Wv_s (natural).
           Inputs arrive as ~14 large DMAs (host pre-lays DRAM so each
           partition's slice is one contiguous run) instead of 162 small
           ones -- dma_start issue costs ~630ns each on the issuing queue.
  phase 2: per (head, q-chunk): S^T[k,q] = K^T.T @ Q^T into a 2-bank
           [128,1024] PSUM tile (two matmuls), ONE 1024-wide ACT exp ->
           P^T f16 in SBUF (wide exps amortize the 352-cycle ACT fixed
           cost), DVE accumulates P^T tiles in f16 (2x DVE mode), and two
           accumulating matmuls per group stream V-lhsT -> unnormalized
           O^T. The softmax denominator broadcast is a ones-lhsT matmul
           over the f16 accumulator (no GPSIMD -- its partition_all_reduce
           took 3.6us and stalled DVE via the shared SBUF port), then DVE
           reciprocal+multiply normalizes.
  phase 3: out_partial[q,e] = O^T.T @ Wo_s; its matmul groups are
           interleaved one-per-attention-group into phase 2 so the PE
           stays dense (no HAM re-throttle) and the tail is short.
           Output is written f16 (host sums partials in f64).
"""

import sys

sys.path.insert(0, "/opt/trn_rl_repo")

import numpy as np

S = 2048
D = 2048
H = 16
DK = 128
N_CORES = 8
HEADS_PER_CORE = H // N_CORES  # 2
DPC = HEADS_PER_CORE * DK  # 256, per-core projection width
SCALE = 1.0 / np.sqrt(DK)

TRACE = False  # test.py flips this to get an NTFF profile + exec time
_LAST_EXEC_NS = [None]
_LAST_RESULTS = [None]

_PROGRAM = [None]


def _build_program():
    from concourse import bacc, mybir
    from concourse.tile import TileContext

    f32 = mybir.dt.float32
    f16 = mybir.dt.float16

    nc = bacc.Bacc()
    # host-side layouts (contiguous per-partition runs for fat DMA descriptors):
    #   xt4[sc, p, dt*512+s] : x^T chunk sc, 16KB runs
    #   wq/wk/wv[p, dt*256+n]: 8KB runs     wo[p, t*2048+e]: 8KB runs
    #   out[qt, p, e]        : 4KB runs
    xt4 = nc.declare_dram_parameter("xt4", [4, 128, 16 * 512], f16, isOutput=False)
    wq = nc.declare_dram_parameter("wq", [128, 16 * DPC], f16, isOutput=False)
    wk = nc.declare_dram_parameter("wk", [128, 16 * DPC], f16, isOutput=False)
    wv = nc.declare_dram_parameter("wv", [128, 16 * DPC], f16, isOutput=False)
    wo = nc.declare_dram_parameter("wo", [128, 2 * S], f16, isOutput=False)
    out = nc.declare_dram_parameter("out", [16, 128, S], f16, isOutput=True)

    ND = D // 128  # 16 d-tiles of the model dim
    NS = S // 128  # 16 s-tiles
    NQ = S // 512  # 4 q/s chunks
    EXP = mybir.ActivationFunctionType.Exp
    CPY = mybir.ActivationFunctionType.Copy

    with TileContext(nc) as tc:
        with (
            tc.tile_pool(name="wpool", bufs=1) as wpool,
            tc.tile_pool(name="xpool", bufs=3) as xpool,
            tc.tile_pool(name="qkv", bufs=1) as qkv,
            tc.tile_pool(name="ppool", bufs=3) as ppool,
            tc.tile_pool(name="apool", bufs=2) as apool,
            tc.tile_pool(name="rpool", bufs=2) as rpool,
            tc.tile_pool(name="opool", bufs=8) as opool,
            tc.tile_pool(name="obpool", bufs=4) as obpool,
            tc.tile_pool(name="psA", bufs=2, space="PSUM") as psA,
            tc.tile_pool(name="psB", bufs=2, space="PSUM") as psB,
            tc.tile_pool(name="psC", bufs=2, space="PSUM") as psC,
        ):
            wq_sb = wpool.tile([128, ND * DPC], f16, tag="wq")
            wk_sb = wpool.tile([128, ND * DPC], f16, tag="wk")
            wv_sb = wpool.tile([128, ND * DPC], f16, tag="wv")
            ones_sb = wpool.tile([128, 128], f16, tag="ones")
            nc.vector.memset(ones_sb[:], 1.0)

            # per-head Q^T/K^T [128, S] and V in natural layout [128, NS*DPC]
            qt_sb = [qkv.tile([128, S], f16, tag=f"qt{h}", name=f"qt{h}") for h in range(2)]
            kt_sb = [qkv.tile([128, S], f16, tag=f"kt{h}", name=f"kt{h}") for h in range(2)]
            v_sb = qkv.tile([128, NS * DPC], f16, tag="v")

            # preload the Exp ACT table while the pipeline fills (one-time
            # ~2.7us table DMA that would otherwise stall the first real exp)
            warm = rpool.tile([128, 8], f32, tag="warm", bufs=1)
            nc.scalar.activation(warm[:], ones_sb[:, 0:8], EXP)

            # ---- input DMA issue ----
            # critical stream on the sync queue: wq/x0 quarters interleaved,
            # then wk, wv.  bulk prefetch (x1-3, wo) on the scalar queue
            # (idle in phase 1); x3/wo block that queue until their slot
            # retires, which is long before phase 2 needs ACT.
            xts = [xpool.tile([128, ND * 512], f16, tag="xt", name=f"xt{sc}") for sc in range(NQ)]
            for qd in range(4):
                nc.sync.dma_start(
                    out=wq_sb[:, qd * 1024:(qd + 1) * 1024],
                    in_=wq[:, qd * 1024:(qd + 1) * 1024],
                )
                nc.sync.dma_start(
                    out=xts[0][:, qd * 2048:(qd + 1) * 2048],
                    in_=xt4[0, :, qd * 2048:(qd + 1) * 2048],
                )
            nc.sync.dma_start(out=wk_sb[:], in_=wk[:])
            nc.sync.dma_start(out=wv_sb[:], in_=wv[:])
            for sc in range(1, NQ):
                nc.scalar.dma_start(out=xts[sc][:], in_=xt4[sc, :, :])
            wo_ot = xpool.tile([128, ND * 512], f16, tag="xt", name="wo_ot")
            wo_sb = wo_ot[:, 0:2 * S]
            nc.scalar.dma_start(out=wo_sb, in_=wo[:])

            # ---------------- phase 1: projections ----------------
            for sc in range(NQ):  # 4 chunks of 512 seq positions
                xt = xts[sc]
                # chunk 3 computes K^T/V first so attention can start
                # before its Q^T (only needed by the last q-chunk) is done
                wlist = ((wq_sb, qt_sb), (wk_sb, kt_sb))
                if sc == NQ - 1:
                    wlist = ((wk_sb, kt_sb), (wq_sb, qt_sb))

                def emit_v():
                    # V natural: [s_tile 128, 256] = sum_d xT[d, s_tile].T @ Wv[d, :]
                    for st in range(4):
                        s_tile = sc * 4 + st
                        ps = psA.tile([128, 512], f32, tag="proj", name="v_ps")
                        for dt_ in range(ND):
                            nc.tensor.matmul(
                                ps[:, 0:DPC],
                                xt[:, dt_ * 512 + st * 128: dt_ * 512 + st * 128 + 128],
                                wv_sb[:, dt_ * DPC:(dt_ + 1) * DPC],
                                start=(dt_ == 0),
                                stop=(dt_ == ND - 1),
                            )
                        nc.vector.tensor_copy(
                            v_sb[:, s_tile * DPC:(s_tile + 1) * DPC], ps[:, 0:DPC]
                        )

                first = True
                for w_sb, dst in wlist:
                    # Q^T/K^T: [n_tile 128, s 512] = sum_d W[d, n].T @ xT[d, s]
                    for h in range(2):
                        ps = psA.tile([128, 512], f32, tag="proj", name="qk_ps")
                        for dt_ in range(ND):
                            nc.tensor.matmul(
                                ps[:],
                                w_sb[:, dt_ * DPC + h * 128: dt_ * DPC + h * 128 + 128],
                                xt[:, dt_ * 512:(dt_ + 1) * 512],
                                start=(dt_ == 0),
                                stop=(dt_ == ND - 1),
                            )
                        nc.vector.tensor_copy(
                            dst[h][:, sc * 512:(sc + 1) * 512], ps[:]
                        )
                    if sc == NQ - 1 and first:
                        emit_v()  # last chunk: V right after K^T
                    first = False
                if sc != NQ - 1:
                    emit_v()

            # ------- phases 2+3 interleaved -------
            # attention per (q-chunk, head); the output projection for
            # q-chunk qc is emitted one group per attention-group during the
            # next two iterations, keeping the PE dense and the tail short.
            ot_tiles = {}
            pending = []  # (qc, qt_local, ec) proj groups ready to emit

            def emit_proj_group():
                qc_, qt_, ec = pending.pop(0)
                ps = psA.tile([128, 512], f32, tag="proj", name="proj_ps")
                for dt_ in range(2):
                    nc.tensor.matmul(
                        ps[:],
                        ot_tiles[(dt_, qc_)][:, qt_ * 128:(qt_ + 1) * 128],
                        wo_sb[:, dt_ * S + ec * 512:dt_ * S + ec * 512 + 512],
                        start=(dt_ == 0),
                        stop=(dt_ == 1),
                    )
                ob = ob_tiles[(qc_, qt_)]
                if ec % 2 == 0:
                    nc.scalar.activation(ob[:, ec * 512:(ec + 1) * 512], ps[:], CPY)
                else:
                    nc.vector.tensor_copy(ob[:, ec * 512:(ec + 1) * 512], ps[:])
                if ec == NQ - 1:
                    nc.sync.dma_start(out=out[qc_ * 4 + qt_], in_=ob[:])

            ob_tiles = {}
            for qc in range(NQ):
                for h in range(2):
                    oT = psB.tile([128, 512], f32, tag="oT", name="oT")
                    qt_slice = qt_sb[h][:, qc * 512:(qc + 1) * 512]
                    acc = apool.tile([128, 1024], f16, tag="acc", name="acc")
                    st = psC.tile([128, 1024], f32, tag="st", name="st0")
                    for half in range(2):
                        nc.tensor.matmul(
                            st[:, half * 512:(half + 1) * 512],
                            kt_sb[h][:, half * 128:(half + 1) * 128],
                            qt_slice,
                            start=True, stop=True,
                        )
                    for g in range(8):  # 8 groups of 2 k-tiles
                        pt = ppool.tile([128, 1024], f16, tag="pt", name="pt")
                        nc.scalar.activation(pt[:], st[:], EXP, scale=float(SCALE))
                        if g + 1 < 8:
                            st = psC.tile([128, 1024], f32, tag="st", name="stn")
                            for half in range(2):
                                kt_ = (g + 1) * 2 + half
                                nc.tensor.matmul(
                                    st[:, half * 512:(half + 1) * 512],
                                    kt_sb[h][:, kt_ * 128:(kt_ + 1) * 128],
                                    qt_slice,
                                    start=True, stop=True,
                                )
                        if g == 0:
                            nc.vector.tensor_copy(acc[:], pt[:])
                        else:
                            nc.vector.tensor_add(acc[:], acc[:], pt[:])
                        for half in range(2):
                            kt_ = g * 2 + half
                            nc.tensor.matmul(
                                oT[:],
                                v_sb[:, kt_ * DPC + h * 128: kt_ * DPC + h * 128 + 128],
                                pt[:, half * 512:(half + 1) * 512],
                                start=(kt_ == 0), stop=(kt_ == NS - 1),
                            )
                        if pending:
                            emit_proj_group()
                    # softmax denominator broadcast to all partitions via a
                    # ones-lhsT matmul over the f16 accumulator
                    bc = psA.tile([128, 512], f32, tag="proj", name="bc")
                    nc.tensor.matmul(bc[:], ones_sb[:], acc[:, 0:512],
                                     start=True, stop=False)
                    nc.tensor.matmul(bc[:], ones_sb[:], acc[:, 512:1024],
                                     start=False, stop=True)
                    rc = rpool.tile([128, 512], f32, tag="recip", name="rc")
                    # ~18-bit 1/x, ~5x faster than exact reciprocal; softmax
                    # denominators are well-conditioned positives (~1e2..4e3)
                    nc.vector.reciprocal_approx_fast(rc[:], bc[:])
                    ot_tiles[(h, qc)] = opool.tile(
                        [128, 512], f16, tag="ot", name="ot_t"
                    )
                    nc.vector.tensor_mul(ot_tiles[(h, qc)][:], oT[:], rc[:])
                # both heads of qc done: queue its 16 proj groups (qt-major
                # so each out row's 4 copies finish before its DMA)
                for qt_ in range(4):
                    ob_tiles[(qc, qt_)] = obpool.tile(
                        [128, S], f16, tag="ob", name="ob"
                    )
                    for ec in range(NQ):
                        pending.append((qc, qt_, ec))
            while pending:
                emit_proj_group()

    nc.compile()
    return nc


def _numpy_fallback(x, mask, Wq, bq, Wk, bk, Wv, bv, Wo, bo):
    B, S_, D_ = x.shape
    xf = x.reshape(S_, D_).astype(np.float64)

    def proj(W, b):
        y = xf @ W.astype(np.float64) + b.astype(np.float64)
        return y.reshape(S_, H, DK).transpose(1, 0, 2)

    Q = proj(Wq, bq)
    K = proj(Wk, bk)
    V = proj(Wv, bv)
    m = np.broadcast_to(mask, (B, H, S_, S_))
    out = np.empty((H, S_, DK))
    for h in range(H):
        sc = Q[h] @ K[h].T / np.sqrt(DK)
        sc = np.where(m[0, h], sc, -np.inf)
        sc -= sc.max(axis=-1, keepdims=True)
        e = np.exp(sc)
        p = e / e.sum(axis=-1, keepdims=True)
        out[h] = p @ V[h]
    o = out.transpose(1, 0, 2).reshape(S_, D_)
    res = o @ Wo.astype(np.float64) + bo.astype(np.float64)
    return res.reshape(B, S_, D_).astype(np.float32)


def kernel(x, mask, Wq, bq, Wk, bk, Wv, bv, Wo, bo):
    x = np.asarray(x, dtype=np.float32)
    mask = np.asarray(mask)
    Wq = np.asarray(Wq, dtype=np.float32)
    Wk = np.asarray(Wk, dtype=np.float32)
    Wv = np.asarray(Wv, dtype=np.float32)
    Wo = np.asarray(Wo, dtype=np.float32)
    bq = np.asarray(bq, dtype=np.float32)
    bk = np.asarray(bk, dtype=np.float32)
    bv = np.asarray(bv, dtype=np.float32)
    bo = np.asarray(bo, dtype=np.float32)

    # Off-benchmark shapes/masks/biases: exact numpy fallback.
    # (bk shifts every score row by a constant -> softmax-invariant; bv and bo
    # are affine in the output and folded in on the host; only bq actually
    # changes the attention pattern in a way the device kernel doesn't model.)
    if x.shape != (1, S, D) or not bool(mask.all()) or np.any(bq):
        return _numpy_fallback(x, mask, Wq, bq, Wk, bk, Wv, bv, Wo, bo)

    from concourse.bass_utils import run_bass_kernel_spmd

    if _PROGRAM[0] is None:
        _PROGRAM[0] = _build_program()
    nc = _PROGRAM[0]

    # xT[dt*128+p, sc*512+s] -> [sc, p, dt, s] (16KB contiguous per partition)
    xT = x.reshape(S, D).T.astype(np.float16)
    xt4 = np.ascontiguousarray(
        xT.reshape(16, 128, 4, 512).transpose(2, 1, 0, 3)
    ).reshape(4, 128, 16 * 512)

    def wlay(Wcol):  # [dt*128+p, n] -> [p, dt*n]
        return np.ascontiguousarray(
            Wcol.reshape(16, 128, DPC).transpose(1, 0, 2)
        ).reshape(128, 16 * DPC)

    in_maps = []
    for c in range(N_CORES):
        lo, hi = c * DPC, (c + 1) * DPC
        wo_c = np.ascontiguousarray(
            Wo[lo:hi, :].astype(np.float16).reshape(2, 128, S).transpose(1, 0, 2)
        ).reshape(128, 2 * S)
        in_maps.append(
            {
                "xt4": xt4,
                "wq": wlay(Wq[:, lo:hi].astype(np.float16)),
                "wk": wlay(Wk[:, lo:hi].astype(np.float16)),
                "wv": wlay(Wv[:, lo:hi].astype(np.float16)),
                "wo": wo_c,
            }
        )

    res = run_bass_kernel_spmd(nc, in_maps, list(range(N_CORES)), trace=TRACE)
    _LAST_EXEC_NS[0] = res.exec_time_ns
    _LAST_RESULTS[0] = res

    acc = res.results[0]["out"].astype(np.float64)
    for c in range(1, N_CORES):
        acc += res.results[c]["out"]
    acc = acc.reshape(S, D)
    # bv contributes (attn rows sum to 1) a constant bv @ Wo; bo is additive.
    acc += (bv.astype(np.float64) @ Wo) + bo
    return acc.astype(np.float32).reshape(1, S, D)
